# revision 10
# baseline (speedup 1.0000x reference)
"""Causal multi-head attention (B=2, S=2048, D=1024, H=16, Dh=64) on 8 TRN2 cores.

Sharding: core c -> batch b=c//4, head-group g=c%4 (heads 4g..4g+3, d_out cols
g*256..(g+1)*256). Each core computes Q/K/V projections for its head group from
x[b] and runs causal attention for its 4 heads independently. No collectives.

Per-core dataflow:
  phase A: load x[b]^T (pre-transposed on host) + W slices; PE computes
           Q^T,K^T (f32, [d_out, S] layout) and V (fp16, [S, d_out] layout).
  phase B: per (q-tile, head): S row = Q_h^T.T @ K_h^T chunks into PSUM (f32),
           additive causal mask on the diagonal block, row-max (DVE, negated),
           exp(S - max) on ACT with per-partition bias + accumulated row-sum,
           P row (fp16) block-transposed via DMA xbar, P^T @ V accumulated on
           PE, then per-partition 1/l scaling.
"""

import math

import numpy as np

B = 2
SEQ = 2048
DIN = 1024
H = 16
DH = 64
NCORES = 8
DO = 256  # d_out columns per core (4 heads)
HPC = 4  # heads per core
KT_N = DIN // 128  # 8 contraction tiles
ST_N = SEQ // 128  # 16 seq tiles
NEG = -1.0e9
SUB = 1024  # S-row PSUM subtile length (2 banks)

_CACHE = {}
LAST_RESULTS = None


def _emit_core_kernel(tc, outs, ins):
    import concourse.bass as bass
    from concourse import mybir

    nc = tc.nc
    f32 = mybir.dt.float32
    f16 = mybir.dt.float16
    (out,) = outs
    xT, wq, wk, wv, mask = ins

    from contextlib import ExitStack

    with ExitStack() as ctx:
        consts = ctx.enter_context(tc.tile_pool(name="consts", bufs=1))
        proj_out = ctx.enter_context(tc.tile_pool(name="proj_out", bufs=1))
        prow_pool = ctx.enter_context(tc.tile_pool(name="prow", bufs=3))
        ptrow_pool = ctx.enter_context(tc.tile_pool(name="ptrow", bufs=3))
        stats = ctx.enter_context(tc.tile_pool(name="stats", bufs=6))
        outp = ctx.enter_context(tc.tile_pool(name="outp", bufs=2))

        # ---- load inputs ----
        mask_sb = consts.tile([128, 128], f32, tag="mask")
        nc.sync.dma_start(mask_sb[:], mask[:])

        xt_sb = []
        for k in range(KT_N):
            t = consts.tile([128, SEQ], f32, tag=f"xt{k}", name=f"xt{k}")
            nc.sync.dma_start(t[:], xT[k * 128 : (k + 1) * 128, :])
            xt_sb.append(t)

        w_sb = {}
        for wname, wap in (("wq", wq), ("wk", wk), ("wv", wv)):
            t = consts.tile([128, KT_N, DO], f32, tag=wname, name=f"{wname}_sb")
            # W is [DIN, DO] row-major; tile block k holds rows k*128..+128
            nc.sync.dma_start(
                t[:], wap.rearrange("(k p) n -> p k n", p=128)
            )
            w_sb[wname] = t

        # ---- phase A: projections ----
        qt_sb = [
            proj_out.tile([128, SEQ], f32, tag=f"qt{m}", name=f"qt{m}")
            for m in range(2)
        ]
        kt_sb = [
            proj_out.tile([128, SEQ], f32, tag=f"kt{m}", name=f"kt{m}")
            for m in range(2)
        ]
        v_sb = [
            proj_out.tile([128, DO], f16, tag=f"v{s}", name=f"v{s}")
            for s in range(ST_N)
        ]

        with tc.tile_pool(name="ps_proj", bufs=4, space="PSUM") as ps_proj:
            # Q^T / K^T: [d_out 128-half, s 512-chunk] = sum_k W[k,m-half].T @ xT[k, chunk]
            for wname, dst in (("wq", qt_sb), ("wk", kt_sb)):
                for m in range(2):
                    for sc in range(SEQ // 512):
                        pst = ps_proj.tile([128, 512], f32, tag="pproj")
                        for k in range(KT_N):
                            nc.tensor.matmul(
                                pst[:],
                                w_sb[wname][:, k, m * 128 : (m + 1) * 128],
                                xt_sb[k][:, sc * 512 : (sc + 1) * 512],
                                start=(k == 0),
                                stop=(k == KT_N - 1),
                            )
                        nc.scalar.copy(dst[m][:, sc * 512 : (sc + 1) * 512], pst[:])
            # V: [s-tile, d_out 256] = sum_k xT[k, s-tile].T @ Wv[k]
            for st in range(ST_N):
                psv = ps_proj.tile([128, DO], f32, tag="pv")
                for k in range(KT_N):
                    nc.tensor.matmul(
                        psv[:],
                        xt_sb[k][:, st * 128 : (st + 1) * 128],
                        w_sb["wv"][:, k, :],
                        start=(k == 0),
                        stop=(k == KT_N - 1),
                    )
                nc.vector.tensor_copy(v_sb[st][:], psv[:])

        # ---- phase B: attention ----
        with (
            tc.tile_pool(name="ps_s", bufs=3, space="PSUM") as ps_s,
            tc.tile_pool(name="ps_o", bufs=2, space="PSUM") as ps_o,
        ):
            for qt in range(ST_N):
                L = (qt + 1) * 128
                out_stage = outp.tile([128, DO], f32, tag="ostage", name="ostage")
                for h in range(HPC):
                    m2, poff = h // 2, (h % 2) * 64
                    lhsT_q = qt_sb[m2][poff : poff + 64, qt * 128 : (qt + 1) * 128]
                    subs = [(0, min(L, SUB))]
                    if L > SUB:
                        subs.append((SUB, L - SUB))
                    mneg_parts = stats.tile([128, 2], f32, tag="mneg_p", name="mneg_p")
                    ps_tiles = []
                    for si, (off, ls) in enumerate(subs):
                        ps = ps_s.tile([128, SUB], f32, tag="srow", name="srow")
                        ps_tiles.append((ps, off, ls))
                        for c0 in range(0, ls, 512):
                            c1 = min(ls, c0 + 512)
                            nc.tensor.matmul(
                                ps[:, c0:c1],
                                lhsT_q,
                                kt_sb[m2][poff : poff + 64, off + c0 : off + c1],
                                start=True,
                                stop=True,
                            )
                        if off + ls == L:
                            # causal mask on the diagonal 128-block
                            nc.vector.tensor_add(
                                ps[:, ls - 128 : ls], ps[:, ls - 128 : ls], mask_sb[:]
                            )
                        nc.vector.reduce_max(
                            mneg_parts[:, si : si + 1],
                            ps[:, :ls],
                            axis=mybir.AxisListType.X,
                            negate=True,
                        )
                    if len(subs) == 2:
                        mneg = stats.tile([128, 1], f32, tag="mneg", name="mneg")
                        nc.vector.tensor_reduce(
                            mneg[:, 0:1],
                            mneg_parts[:, 0:2],
                            axis=mybir.AxisListType.X,
                            op=mybir.AluOpType.min,
                        )
                        mneg_ap = mneg[:, 0:1]
                    else:
                        mneg_ap = mneg_parts[:, 0:1]

                    p_row = prow_pool.tile([128, SEQ], f16, tag="prow", name="prow")
                    lparts = stats.tile([128, 2], f32, tag="lparts", name="lparts")
                    for si, (ps, off, ls) in enumerate(ps_tiles):
                        nc.scalar.activation(
                            p_row[:, off : off + ls],
                            ps[:, :ls],
                            mybir.ActivationFunctionType.Exp,
                            bias=mneg_ap,
                            scale=1.0,
                            accum_out=lparts[:, si : si + 1],
                        )
                    if len(subs) == 2:
                        lsum = stats.tile([128, 1], f32, tag="lsum", name="lsum")
                        nc.vector.reduce_sum(
                            lsum[:, 0:1], lparts[:, 0:2], axis=mybir.AxisListType.X
                        )
                        lsum_ap = lsum[:, 0:1]
                    else:
                        lsum_ap = lparts[:, 0:1]
                    r = stats.tile([128, 1], f32, tag="r", name="r")
                    nc.vector.reciprocal(r[:, 0:1], lsum_ap)

                    pt_row = ptrow_pool.tile([128, ST_N, 128], f16, tag="ptrow", name="ptrow")
                    nc.sync.dma_start_transpose(
                        pt_row[:, : qt + 1, :], p_row[:, :L]
                    )

                    po = ps_o.tile([128, DH], f32, tag="po", name="po")
                    for kt in range(qt + 1):
                        nc.tensor.matmul(
                            po[:],
                            pt_row[:, kt, :],
                            v_sb[kt][:, h * DH : (h + 1) * DH],
                            start=(kt == 0),
                            stop=(kt == qt),
                        )
                    nc.vector.tensor_scalar_mul(
                        out_stage[:, h * DH : (h + 1) * DH], po[:], r[:, 0:1]
                    )
                nc.sync.dma_start(out[qt * 128 : (qt + 1) * 128, :], out_stage[:])


def _split_waits(nc):
    """This container's walrus accepts at most ONE sync-wait on Matmult/Drain
    instructions ("Too many sync wait commands" otherwise). Hoist excess waits
    into standalone InstEventSemaphore instructions on the same engine."""
    from concourse import mybir

    cap = 1
    n = 0
    for f in nc.m.functions:
        for bb in f.blocks:
            new = []
            for inst in list(bb.instructions):
                si = inst.sync_info
                waits = list(si.on_wait) if si is not None else []
                if len(waits) > cap:
                    for j, w in enumerate(waits[cap:]):
                        new.append(
                            mybir.InstEventSemaphore(
                                name=f"{inst.name}-w{j}",
                                engine=inst.engine,
                                ins=[],
                                outs=[],
                                sync_info=mybir.SyncInfo(on_wait=[w], on_update=[]),
                            )
                        )
                        n += 1
                    inst.sync_info = mybir.SyncInfo(
                        on_wait=waits[:cap], on_update=list(si.on_update)
                    )
                new.append(inst)
            bb.instructions = new
    return n


def _build_nc():
    import concourse.bass as bass
    import concourse.tile as tile
    from concourse import mybir

    f32 = mybir.dt.float32
    nc = bass.Bass(
        "TRN2",
        target_bir_lowering=False,
        debug=False,
        num_devices=NCORES,
    )
    xT = nc.dram_tensor("xT", [DIN, SEQ], f32, kind="ExternalInput").ap()
    wq = nc.dram_tensor("wq", [DIN, DO], f32, kind="ExternalInput").ap()
    wk = nc.dram_tensor("wk", [DIN, DO], f32, kind="ExternalInput").ap()
    wv = nc.dram_tensor("wv", [DIN, DO], f32, kind="ExternalInput").ap()
    mask = nc.dram_tensor("mask", [128, 128], f32, kind="ExternalInput").ap()
    out = nc.dram_tensor("out", [SEQ, DO], f32, kind="ExternalOutput").ap()

    with tile.TileContext(nc) as tc:
        _emit_core_kernel(tc, (out,), (xT, wq, wk, wv, mask))
    _split_waits(nc)
    return nc


def make_mask():
    m = np.zeros((128, 128), dtype=np.float32)
    q = np.arange(128)[:, None]
    k = np.arange(128)[None, :]
    m[k > q] = NEG
    return m


def shard_inputs(x, W_q, W_k, W_v):
    x = np.asarray(x, dtype=np.float32)
    W_q = np.asarray(W_q, dtype=np.float32)
    W_k = np.asarray(W_k, dtype=np.float32)
    W_v = np.asarray(W_v, dtype=np.float32)
    mask = make_mask()
    scale = 1.0 / math.sqrt(DH)
    in_maps = []
    for c in range(NCORES):
        b, g = divmod(c, NCORES // B)
        sl = slice(g * DO, (g + 1) * DO)
        in_maps.append(
            {
                "xT": np.ascontiguousarray(x[b].T),
                "wq": np.ascontiguousarray(W_q[:, sl] * scale),
                "wk": np.ascontiguousarray(W_k[:, sl]),
                "wv": np.ascontiguousarray(W_v[:, sl]),
                "mask": mask,
            }
        )
    return in_maps


def _install_axon_ntff_hook():
    """Provide antenv.axon_hooks (missing in this image) so trace=True works
    under axon. Mirrors trn_agent_boot.trn_boot._ntff_profile_via_ctypes."""
    import contextlib
    import ctypes
    import sys
    import types

    if "antenv.axon_hooks" in sys.modules:
        return True
    try:
        lib = ctypes.CDLL("/opt/axon/libaxon_pjrt.so")
    except OSError:
        return False
    if not hasattr(lib, "axon_start_nrt_profile"):
        return False
    lib.axon_start_nrt_profile.argtypes = [
        ctypes.POINTER(ctypes.c_int64),
        ctypes.c_size_t,
    ]
    lib.axon_start_nrt_profile.restype = ctypes.c_int64
    lib.axon_stop_nrt_profile.argtypes = [ctypes.c_char_p]
    lib.axon_stop_nrt_profile.restype = ctypes.c_int64

    @contextlib.contextmanager
    def _hook(output_dir, device_ids):
        import jax

        jax.devices()
        if device_ids:
            ids = (ctypes.c_int64 * len(device_ids))(*device_ids)
            rc = lib.axon_start_nrt_profile(ids, len(device_ids))
        else:
            rc = lib.axon_start_nrt_profile(None, 0)
        if rc != 0:
            raise RuntimeError(f"axon_start_nrt_profile rc={rc}")
        try:
            yield
        finally:
            n = lib.axon_stop_nrt_profile(str(output_dir).encode())
            print(f"ntff profile: {n} file(s) written to {output_dir}")

    mod = types.ModuleType("antenv.axon_hooks")
    holder = [_hook]
    mod.get_axon_ntff_profile_hook = lambda: holder[0]
    mod.set_axon_ntff_profile_hook = lambda h: holder.__setitem__(0, h)
    sys.modules["antenv.axon_hooks"] = mod
    import antenv

    antenv.axon_hooks = mod
    return True


def kernel(x, W_q, W_k, W_v):
    global LAST_RESULTS
    import concourse.bass_utils as bass_utils
    from concourse.bass_utils import run_bass_kernel_spmd

    if "nc" not in _CACHE:
        _CACHE["nc"] = _build_nc()
    nc = _CACHE["nc"]

    in_maps = shard_inputs(x, W_q, W_k, W_v)
    import os

    trace = bool(int(os.environ.get("MHA_TRACE", "0")))
    if trace:
        trace = _install_axon_ntff_hook()
        # avoid the fish-bucket artifact upload in this container
        bass_utils.upload_artifacts = lambda d: str(d)
    res = run_bass_kernel_spmd(
        nc, in_maps, core_ids=list(range(NCORES)), trace=trace
    )
    LAST_RESULTS = res

    out = np.zeros((B, SEQ, DIN), dtype=np.float32)
    for c in range(NCORES):
        b, g = divmod(c, NCORES // B)
        out[b, :, g * DO : (g + 1) * DO] = res.results[c]["out"]
    return out


# revision 17
# speedup vs baseline: 1.0911x; 1.0911x over previous
"""Causal multi-head attention (B=2, S=2048, D=1024, H=16, Dh=64) on 8 TRN2 cores.

Sharding: core c -> batch b=c//4, head-group g=c%4 (heads 4g..4g+3, d_out cols
g*256..(g+1)*256). Each core computes Q/K/V projections for its head group from
x[b] and runs causal attention for its 4 heads independently. No collectives.

Per-core dataflow:
  phase A: load x[b]^T (pre-transposed on host) + W slices; PE computes
           Q^T,K^T (f32, head-pair layout [128, S]) and V+ones (fp16,
           [S, 4*65] interleaved per head).
  phase B: per (q-tile, head): S row chunks = Q_h^T.T @ K_h^T into PSUM (f32,
           two heads packed into PE row-groups 0-63 / 64-127), additive causal
           mask on the diagonal block (DVE), row-max (DVE reduce, negated),
           exp(S - max) on ACT (per-partition bias) -> P row fp16, batched
           128-block transpose via DMA xbar into per-(head, q-chunk) k-major
           tiles, then O^T[65, 512] = sum_kt V~[kt].T @ P^T[kt] on PE (fp16,
           row 64 = softmax denominator via the ones column).
  host:    out = (O^T[:64] / O^T[64]) transposed back, assembled across cores.
"""

import math

import numpy as np

B = 2
SEQ = 2048
DIN = 1024
H = 16
DH = 64
NCORES = 8
DO = 256  # d_out columns per core (4 heads)
HPC = 4  # heads per core
KT_N = DIN // 128  # 8 contraction tiles
ST_N = SEQ // 128  # 16 seq tiles
QC_N = SEQ // 512  # 4 q-chunks for PV
NEG = -1.0e9
SUB = 1024  # S-row PSUM subtile length (2 banks)

_CACHE = {}
LAST_RESULTS = None


def _emit_core_kernel(tc, outs, ins):
    from concourse import mybir

    nc = tc.nc
    f32 = mybir.dt.float32
    f16 = mybir.dt.float16
    (outT,) = outs  # [HPC, 65, SEQ] f32
    xT, wq, wk, wv, mask = ins

    from contextlib import ExitStack

    with ExitStack() as ctx:
        consts = ctx.enter_context(tc.tile_pool(name="consts", bufs=1))
        proj_out = ctx.enter_context(tc.tile_pool(name="proj_out", bufs=1))

        mask_sb = consts.tile([128, 128], f32, tag="mask")
        nc.sync.dma_start(mask_sb[:], mask[:])

        # Q^T/K^T in head-pair layout; V in [S, 4*65] fp16 (ones col per head)
        qt_sb = [
            proj_out.tile([128, SEQ], f32, tag=f"qt{m}", name=f"qt{m}")
            for m in range(2)
        ]
        kt_sb = [
            proj_out.tile([128, SEQ], f32, tag=f"kt{m}", name=f"kt{m}")
            for m in range(2)
        ]
        v_sb = [
            proj_out.tile([128, HPC * (DH + 1)], f16, tag=f"v{s}", name=f"v{s}")
            for s in range(ST_N)
        ]

        with ExitStack() as phase_a:
            xt_pool = phase_a.enter_context(tc.tile_pool(name="xt", bufs=1))
            ps_proj = phase_a.enter_context(
                tc.tile_pool(name="ps_proj", bufs=4, space="PSUM")
            )
            w_sb = {}
            for wname, wap in (("wq", wq), ("wk", wk), ("wv", wv)):
                t = xt_pool.tile([128, KT_N, DO], f32, tag=wname, name=f"{wname}_sb")
                nc.sync.dma_start(t[:], wap.rearrange("(k p) n -> p k n", p=128))
                w_sb[wname] = t
            xt_sb = []
            for k in range(KT_N):
                t = xt_pool.tile([128, SEQ], f32, tag=f"xt{k}", name=f"xt{k}")
                nc.sync.dma_start(t[:], xT[k * 128 : (k + 1) * 128, :])
                xt_sb.append(t)

            for wname, dst in (("wq", qt_sb), ("wk", kt_sb)):
                for m in range(2):
                    for sc in range(SEQ // 512):
                        pst = ps_proj.tile([128, 512], f32, tag="pproj")
                        for k in range(KT_N):
                            nc.tensor.matmul(
                                pst[:],
                                w_sb[wname][:, k, m * 128 : (m + 1) * 128],
                                xt_sb[k][:, sc * 512 : (sc + 1) * 512],
                                start=(k == 0),
                                stop=(k == KT_N - 1),
                            )
                        nc.scalar.copy(dst[m][:, sc * 512 : (sc + 1) * 512], pst[:])
            for st in range(ST_N):
                psv = ps_proj.tile([128, DO], f32, tag="pv")
                for k in range(KT_N):
                    nc.tensor.matmul(
                        psv[:],
                        xt_sb[k][:, st * 128 : (st + 1) * 128],
                        w_sb["wv"][:, k, :],
                        start=(k == 0),
                        stop=(k == KT_N - 1),
                    )
                # interleave 4 heads with a ones column: [64 v | 1] per head
                for h in range(HPC):
                    nc.scalar.copy(
                        v_sb[st][:, h * (DH + 1) : h * (DH + 1) + DH],
                        psv[:, h * DH : (h + 1) * DH],
                    )
                    nc.vector.memset(
                        v_sb[st][:, h * (DH + 1) + DH : (h + 1) * (DH + 1)], 1.0
                    )

        # ---- phase B ----
        with (
            tc.tile_pool(name="prow", bufs=4) as prow_pool,
            tc.tile_pool(name="ptrow", bufs=3) as ptrow_pool,
            tc.tile_pool(name="stats", bufs=8) as stats,
            tc.tile_pool(name="outp", bufs=3) as outp,
            tc.tile_pool(name="ps_s", bufs=3, space="PSUM") as ps_s,
            tc.tile_pool(name="ps_o", bufs=2, space="PSUM") as ps_o,
        ):
            # per (q-chunk, head): PT tile [128, ST_N, 512] fp16, filled by the
            # 4 row transposes of that q-chunk, consumed by wide-N PV.
            for qc in range(QC_N):
                pt_tiles = {}
                for h in range(HPC):
                    pt_tiles[h] = ptrow_pool.tile(
                        [128, ST_N, 512], f16, tag=f"pt{h % 2}", name=f"pt{h % 2}"
                    )
                for qt in range(qc * 4, qc * 4 + 4):
                    L = (qt + 1) * 128
                    for h in range(HPC):
                        m2, poff = h // 2, (h % 2) * 64
                        lhsT_q = qt_sb[m2][
                            poff : poff + 64, qt * 128 : (qt + 1) * 128
                        ]
                        subs = [(0, min(L, SUB))]
                        if L > SUB:
                            subs.append((SUB, L - SUB))
                        mneg_parts = stats.tile(
                            [128, 2], f32, tag="mneg_p", name="mneg_p"
                        )
                        ps_tiles = []
                        for si, (off, ls) in enumerate(subs):
                            ps = ps_s.tile([128, SUB], f32, tag="srow", name="srow")
                            ps_tiles.append((ps, off, ls))
                            for c0 in range(0, ls, 512):
                                c1 = min(ls, c0 + 512)
                                nc.tensor.matmul(
                                    ps[:, c0:c1],
                                    lhsT_q,
                                    kt_sb[m2][
                                        poff : poff + 64, off + c0 : off + c1
                                    ],
                                    start=True,
                                    stop=True,
                                )
                            if off + ls == L:
                                nc.vector.tensor_add(
                                    ps[:, ls - 128 : ls],
                                    ps[:, ls - 128 : ls],
                                    mask_sb[:],
                                )
                            nc.vector.reduce_max(
                                mneg_parts[:, si : si + 1],
                                ps[:, :ls],
                                axis=mybir.AxisListType.X,
                                negate=True,
                            )
                        if len(subs) == 2:
                            mneg = stats.tile([128, 1], f32, tag="mneg", name="mneg")
                            nc.vector.tensor_reduce(
                                mneg[:, 0:1],
                                mneg_parts[:, 0:2],
                                axis=mybir.AxisListType.X,
                                op=mybir.AluOpType.min,
                            )
                            mneg_ap = mneg[:, 0:1]
                        else:
                            mneg_ap = mneg_parts[:, 0:1]

                        p_row = prow_pool.tile([128, SEQ], f16, tag="prow", name="prow")
                        for ps, off, ls in ps_tiles:
                            nc.scalar.activation(
                                p_row[:, off : off + ls],
                                ps[:, :ls],
                                mybir.ActivationFunctionType.Exp,
                                bias=mneg_ap,
                                scale=1.0,
                            )
                        # batched 128-block transpose of the whole row into the
                        # strided PT tile; alternate HWDGE rings by head parity
                        eng = nc.sync if h % 2 == 0 else nc.scalar
                        eng.dma_start_transpose(
                            pt_tiles[h][:, : qt + 1, (qt % 4) * 128 : (qt % 4) * 128 + 128],
                            p_row[:, :L],
                        )
                # PV for this q-chunk: O^T[65, 512] per head
                for h in range(HPC):
                    po = ps_o.tile([65, 512], f32, tag="po", name="po")
                    kt_hi = qc * 4 + 3
                    for kt in range(kt_hi + 1):
                        off = max(0, (kt - qc * 4)) * 128
                        nc.tensor.matmul(
                            po[:, off:512],
                            v_sb[kt][:, h * (DH + 1) : (h + 1) * (DH + 1)],
                            pt_tiles[h][:, kt, off:512],
                            start=(kt == 0),
                            stop=(kt == kt_hi),
                        )
                    ot = outp.tile([65, 512], f32, tag="ot", name="ot")
                    nc.scalar.copy(ot[:], po[:])
                    nc.sync.dma_start(
                        outT[h, :, qc * 512 : (qc + 1) * 512], ot[:]
                    )


def _split_waits(nc):
    """This container's walrus accepts at most ONE sync-wait per instruction
    on several opcodes ("Too many sync wait commands"). Hoist excess waits
    into standalone InstEventSemaphore instructions on the same engine."""
    from concourse import mybir

    cap = 1
    n = 0
    for f in nc.m.functions:
        for bb in f.blocks:
            new = []
            for inst in list(bb.instructions):
                si = inst.sync_info
                waits = list(si.on_wait) if si is not None else []
                if len(waits) > cap:
                    for j, w in enumerate(waits[cap:]):
                        new.append(
                            mybir.InstEventSemaphore(
                                name=f"{inst.name}-w{j}",
                                engine=inst.engine,
                                ins=[],
                                outs=[],
                                sync_info=mybir.SyncInfo(on_wait=[w], on_update=[]),
                            )
                        )
                        n += 1
                    inst.sync_info = mybir.SyncInfo(
                        on_wait=waits[:cap], on_update=list(si.on_update)
                    )
                new.append(inst)
            bb.instructions = new
    return n


def _build_nc():
    import concourse.bass as bass
    import concourse.tile as tile
    from concourse import mybir

    f32 = mybir.dt.float32
    nc = bass.Bass(
        "TRN2",
        target_bir_lowering=False,
        debug=False,
        num_devices=NCORES,
    )
    xT = nc.dram_tensor("xT", [DIN, SEQ], f32, kind="ExternalInput").ap()
    wq = nc.dram_tensor("wq", [DIN, DO], f32, kind="ExternalInput").ap()
    wk = nc.dram_tensor("wk", [DIN, DO], f32, kind="ExternalInput").ap()
    wv = nc.dram_tensor("wv", [DIN, DO], f32, kind="ExternalInput").ap()
    mask = nc.dram_tensor("mask", [128, 128], f32, kind="ExternalInput").ap()
    outT = nc.dram_tensor("outT", [HPC, DH + 1, SEQ], f32, kind="ExternalOutput").ap()

    with tile.TileContext(nc) as tc:
        _emit_core_kernel(tc, (outT,), (xT, wq, wk, wv, mask))
    _split_waits(nc)
    return nc


def make_mask():
    m = np.zeros((128, 128), dtype=np.float32)
    q = np.arange(128)[:, None]
    k = np.arange(128)[None, :]
    m[k > q] = NEG
    return m


def shard_inputs(x, W_q, W_k, W_v):
    x = np.asarray(x, dtype=np.float32)
    W_q = np.asarray(W_q, dtype=np.float32)
    W_k = np.asarray(W_k, dtype=np.float32)
    W_v = np.asarray(W_v, dtype=np.float32)
    mask = make_mask()
    scale = 1.0 / math.sqrt(DH)
    in_maps = []
    for c in range(NCORES):
        b, g = divmod(c, NCORES // B)
        sl = slice(g * DO, (g + 1) * DO)
        in_maps.append(
            {
                "xT": np.ascontiguousarray(x[b].T),
                "wq": np.ascontiguousarray(W_q[:, sl] * scale),
                "wk": np.ascontiguousarray(W_k[:, sl]),
                "wv": np.ascontiguousarray(W_v[:, sl]),
                "mask": mask,
            }
        )
    return in_maps


def assemble_output(results):
    out = np.zeros((B, SEQ, DIN), dtype=np.float32)
    for c in range(NCORES):
        b, g = divmod(c, NCORES // B)
        oT = results[c]["outT"]  # [HPC, 65, SEQ]
        for h in range(HPC):
            col = g * DO + h * DH
            out[b, :, col : col + DH] = (oT[h, :DH, :] / oT[h, DH : DH + 1, :]).T
    return out


def _install_axon_ntff_hook():
    """Provide antenv.axon_hooks (missing in this image) so trace=True works
    under axon. Mirrors trn_agent_boot.trn_boot._ntff_profile_via_ctypes."""
    import contextlib
    import ctypes
    import sys
    import types

    if "antenv.axon_hooks" in sys.modules:
        return True
    try:
        lib = ctypes.CDLL("/opt/axon/libaxon_pjrt.so")
    except OSError:
        return False
    if not hasattr(lib, "axon_start_nrt_profile"):
        return False
    lib.axon_start_nrt_profile.argtypes = [
        ctypes.POINTER(ctypes.c_int64),
        ctypes.c_size_t,
    ]
    lib.axon_start_nrt_profile.restype = ctypes.c_int64
    lib.axon_stop_nrt_profile.argtypes = [ctypes.c_char_p]
    lib.axon_stop_nrt_profile.restype = ctypes.c_int64

    @contextlib.contextmanager
    def _hook(output_dir, device_ids):
        import jax

        jax.devices()
        if device_ids:
            ids = (ctypes.c_int64 * len(device_ids))(*device_ids)
            rc = lib.axon_start_nrt_profile(ids, len(device_ids))
        else:
            rc = lib.axon_start_nrt_profile(None, 0)
        if rc != 0:
            raise RuntimeError(f"axon_start_nrt_profile rc={rc}")
        try:
            yield
        finally:
            n = lib.axon_stop_nrt_profile(str(output_dir).encode())
            print(f"ntff profile: {n} file(s) written to {output_dir}")

    mod = types.ModuleType("antenv.axon_hooks")
    holder = [_hook]
    mod.get_axon_ntff_profile_hook = lambda: holder[0]
    mod.set_axon_ntff_profile_hook = lambda h: holder.__setitem__(0, h)
    sys.modules["antenv.axon_hooks"] = mod
    import antenv

    antenv.axon_hooks = mod
    return True


def kernel(x, W_q, W_k, W_v):
    global LAST_RESULTS
    import os

    import concourse.bass_utils as bass_utils
    from concourse.bass_utils import run_bass_kernel_spmd

    if "nc" not in _CACHE:
        _CACHE["nc"] = _build_nc()
    nc = _CACHE["nc"]

    in_maps = shard_inputs(x, W_q, W_k, W_v)

    trace = bool(int(os.environ.get("MHA_TRACE", "0")))
    if trace:
        trace = _install_axon_ntff_hook()
        # avoid the fish-bucket artifact upload in this container
        bass_utils.upload_artifacts = lambda d: str(d)
    res = run_bass_kernel_spmd(
        nc, in_maps, core_ids=list(range(NCORES)), trace=trace
    )
    LAST_RESULTS = res
    return assemble_output(res.results)
